# revision 1
# baseline (speedup 1.0000x reference)
"""GNN message-passing (CompGCN-style edge-softmax) Trainium2 kernel.

Contract: kernel(**inputs) takes FULL unsharded inputs (ent_emb [50000,128] f32,
rel_emb [1000,128] f32, neigh_w [128,128] f32, src/dst/rel_id [600000] int) and
returns the FULL [50000,128] f32 output of:

    comp  = ent_emb[src] * rel_emb[rel_id]
    score = sum(comp * ent_emb[dst], -1)
    alpha = segment_softmax(score, dst)          # grouped by dst
    neigh = segment_sum(comp * alpha[:,None], dst)
    out   = tanh(neigh @ neigh_w)

Sharding: edges are sharded across the 8 cores BY DST RANGE (core c owns dst in
[c*6250,(c+1)*6250)), so segment max/sum are purely core-local and no
collective is needed; ent_emb/rel_emb/neigh_w are replicated.  Within a core,
edges are grouped by 128-node dst blocks; per block the kernel bulk-gathers
ent_emb[src] / rel_emb[rel] / ent_emb[dst] rows with dma_gather, computes
score -> es=exp(score) (segment-max subtraction is skipped: |score| <~ 60 <<
88 so exp cannot overflow, and alpha = es/sum(es) is exact), builds the
weighted one-hot W[e,j] = es_e * (dst_e == j) on DVE, and accumulates
    accT[h,j] += comp_c.T @ W_c      (TensorE, PSUM)
    den[j]    += W_c.T @ ones        (TensorE, PSUM)
then out_block = tanh((accT.T @ neigh_w) / den) and a contiguous DMA out.

dma_gather indices are int16, so ent_emb (50000 rows) src-gathers are split
into a lo (rows < 32768) and hi (rows >= 32768) gather per block, with edges
sorted by src inside each section (ascending HBM addresses).  Padded slots
repeat a real edge but carry dst_oh = 128 which matches no iota column, so
they contribute exactly zero to den/acc.
"""

import numpy as np

N_ENT = 50000
N_REL = 1000
N_EDGES = 600000
H = 128
P = 128
N_CORES = 8
NPC = N_ENT // N_CORES          # nodes per core
LO_ROWS = 32768                 # int16 gather split point

_cache = {}


def _build_program(npc, n_blocks, s_lo, s_hi, n_ent, n_rel, lo_rows):
    """Build the SPMD Bass/Tile program for one core shape."""
    import concourse.bacc as bacc
    import concourse.mybir as mybir
    import concourse.tile as tile

    f32 = mybir.dt.float32
    i16 = mybir.dt.int16
    S = s_lo + s_hi

    nc = bacc.Bacc("TRN2", target_bir_lowering=False, debug=False,
                   num_devices=N_CORES)

    ent = nc.dram_tensor("ent", [n_ent, H], f32, kind="ExternalInput")
    # this core's own node slice (dst rows) — per-core data, static local base
    ent_loc = nc.dram_tensor("ent_loc", [npc, H], f32, kind="ExternalInput")
    relt = nc.dram_tensor("relt", [n_rel, H], f32, kind="ExternalInput")
    w_in = nc.dram_tensor("w", [H, H], f32, kind="ExternalInput")
    iota_in = nc.dram_tensor("iota", [P, P], f32, kind="ExternalInput")
    sgi_in = nc.dram_tensor("src_gi", [P, n_blocks, S * 8], i16,
                            kind="ExternalInput")
    rgi_in = nc.dram_tensor("rel_gi", [P, n_blocks, S * 8], i16,
                            kind="ExternalInput")
    dgi_in = nc.dram_tensor("dst_gi", [P, n_blocks, S * 8], i16,
                            kind="ExternalInput")
    doh_in = nc.dram_tensor("dst_oh", [P, n_blocks, S], f32,
                            kind="ExternalInput")
    out = nc.dram_tensor("out", [npc, H], f32, kind="ExternalOutput")

    import concourse.bass as bass

    def bc(ap, dims):
        # append/insert stride-0 dims: dims is the final [step,count] list
        return bass.AP(ap.tensor, ap.offset, dims)

    with tile.TileContext(nc) as tc:
        with (
            tc.tile_pool(name="const", bufs=1) as constp,
            tc.tile_pool(name="idx", bufs=1) as idxp,
            tc.tile_pool(name="data", bufs=2) as datap,
            tc.tile_pool(name="small", bufs=2) as smallp,
            tc.tile_pool(name="psum", bufs=2, space="PSUM") as psump,
            tc.tile_pool(name="psum1", bufs=2, space="PSUM") as psum1p,
        ):
            iota_t = constp.tile([P, P], f32)
            nc.sync.dma_start(iota_t[:], iota_in[:])
            w_t = constp.tile([H, H], f32)
            nc.sync.dma_start(w_t[:], w_in[:])
            ones_t = constp.tile([P, 1], f32)
            nc.vector.memset(ones_t[:], 1.0)

            sgi_t = idxp.tile([P, n_blocks, S * 8], i16)
            nc.sync.dma_start(sgi_t[:], sgi_in[:])
            rgi_t = idxp.tile([P, n_blocks, S * 8], i16)
            nc.sync.dma_start(rgi_t[:], rgi_in[:])
            dgi_t = idxp.tile([P, n_blocks, S * 8], i16)
            nc.sync.dma_start(dgi_t[:], dgi_in[:])
            doh_t = idxp.tile([P, n_blocks, S], f32)
            nc.sync.dma_start(doh_t[:], doh_in[:])

            for b in range(n_blocks):
                base = b * P
                nodes_b = min(P, npc - base)

                src_rows = datap.tile([P, S, H], f32, tag="src")
                rel_rows = datap.tile([P, S, H], f32, tag="rel")
                dst_rows = datap.tile([P, S, H], f32, tag="dst")
                w_oh = datap.tile([P, S, H], f32, tag="W")

                if s_lo > 0:
                    nc.gpsimd.dma_gather(
                        src_rows[:, 0:s_lo, :], ent[0:lo_rows, :],
                        sgi_t[:, b, 0:s_lo * 8], s_lo * P, s_lo * P, H,
                        single_packet=False)
                if s_hi > 0:
                    nc.gpsimd.dma_gather(
                        src_rows[:, s_lo:S, :], ent[lo_rows:n_ent, :],
                        sgi_t[:, b, s_lo * 8:S * 8], s_hi * P, s_hi * P, H,
                        single_packet=False)
                nc.gpsimd.dma_gather(
                    rel_rows[:, :, :], relt[:, :],
                    rgi_t[:, b, :], S * P, S * P, H, single_packet=False)
                nc.gpsimd.dma_gather(
                    dst_rows[:, :, :], ent_loc[base:base + nodes_b, :],
                    dgi_t[:, b, :], S * P, S * P, H, single_packet=False)

                # comp = ent[src] * rel[rel_id]   (in-place over src_rows)
                nc.vector.tensor_tensor(
                    out=src_rows[:], in0=src_rows[:], in1=rel_rows[:],
                    op=mybir.AluOpType.mult)
                # prod = comp * ent[dst]          (in-place over dst_rows)
                nc.vector.tensor_tensor(
                    out=dst_rows[:], in0=src_rows[:], in1=dst_rows[:],
                    op=mybir.AluOpType.mult)
                score = smallp.tile([P, S], f32, tag="score")
                nc.vector.tensor_reduce(
                    out=score[:], in_=dst_rows[:],
                    axis=mybir.AxisListType.X, op=mybir.AluOpType.add)
                es = smallp.tile([P, S], f32, tag="es")
                nc.scalar.activation(
                    out=es[:], in_=score[:],
                    func=mybir.ActivationFunctionType.Exp)

                # one-hot: W[p, c, j] = (dst_oh[p, c] == j)
                doh_ap = doh_t[:, b, :]
                doh_b = bc(doh_ap, [doh_ap.ap[0], doh_ap.ap[1], [0, H]])
                iota_ap = iota_t[:]
                iota_b = bc(iota_ap, [iota_ap.ap[0], [0, S], iota_ap.ap[1]])
                nc.vector.tensor_tensor(
                    out=w_oh[:], in0=doh_b, in1=iota_b,
                    op=mybir.AluOpType.is_equal)
                # W *= es  (broadcast es over the one-hot columns)
                es_ap = es[:]
                es_b = bc(es_ap, [es_ap.ap[0], es_ap.ap[1], [0, H]])
                nc.vector.tensor_tensor(
                    out=w_oh[:], in0=w_oh[:], in1=es_b,
                    op=mybir.AluOpType.mult)

                # accT[h, j] = sum_c comp_c.T @ W_c
                acct_ps = psump.tile([P, P], f32, tag="accT")
                for c in range(S):
                    nc.tensor.matmul(
                        acct_ps[:], lhsT=src_rows[:, c, :], rhs=w_oh[:, c, :],
                        start=(c == 0), stop=(c == S - 1))
                # den[j] = sum_c W_c.T @ ones
                den_ps = psum1p.tile([P, 1], f32, tag="den")
                for c in range(S):
                    nc.tensor.matmul(
                        den_ps[:], lhsT=w_oh[:, c, :], rhs=ones_t[:],
                        start=(c == 0), stop=(c == S - 1))

                acct_sb = smallp.tile([P, P], f32, tag="acct_sb")
                nc.scalar.copy(acct_sb[:], acct_ps[:])
                den_sb = smallp.tile([P, 1], f32, tag="den_sb")
                nc.vector.tensor_scalar_max(den_sb[:], den_ps[:], 1e-30)
                rden = smallp.tile([P, 1], f32, tag="rden")
                nc.vector.reciprocal(rden[:], den_sb[:])

                out_ps = psump.tile([P, H], f32, tag="out_ps")
                nc.tensor.matmul(out_ps[:], lhsT=acct_sb[:], rhs=w_t[:],
                                 start=True, stop=True)
                out_sb = smallp.tile([P, H], f32, tag="out_sb")
                nc.scalar.activation(
                    out=out_sb[:], in_=out_ps[:],
                    func=mybir.ActivationFunctionType.Tanh, scale=rden[:])
                nc.sync.dma_start(out[base:base + nodes_b, :],
                                  out_sb[:nodes_b, :])

    nc.compile()
    return nc


def _idx_to_gather_layout(arr):
    """[S*128] int16 gather-position-ordered indices -> [128, S*8] tile."""
    a = arr.reshape(-1, 16).T.astype(np.int16)      # [16, S*8]
    return np.tile(a, (8, 1))                        # [128, S*8]


def _prep_inputs(ent_emb, rel_emb, neigh_w, src, dst, rel_id):
    """Partition edges by dst core/block, build per-core gather index arrays.

    Returns (in_maps, shape_key) where shape_key parameterizes the program.
    """
    src = np.asarray(src).astype(np.int64)
    dst = np.asarray(dst).astype(np.int64)
    rel_id = np.asarray(rel_id).astype(np.int64)
    n_blocks = (NPC + P - 1) // P

    order = np.argsort(dst, kind="stable")
    src_s, dst_s, rel_s = src[order], dst[order], rel_id[order]
    # per-(core,block) group id; monotone in dst since blocks nest in cores
    g_s = (dst_s // NPC) * n_blocks + (dst_s % NPC) // P
    n_gblocks = N_CORES * n_blocks
    bounds = np.searchsorted(g_s, np.arange(n_gblocks + 1))

    # first pass: per-block lo/hi counts -> global S_LO / S_HI
    max_lo = 1
    max_hi = 1
    lohi = []
    for g in range(n_gblocks):
        e0, e1 = bounds[g], bounds[g + 1]
        s_g = src_s[e0:e1]
        n_lo = int((s_g < LO_ROWS).sum())
        n_hi = int(e1 - e0 - n_lo)
        lohi.append((e0, e1, n_lo, n_hi))
        max_lo = max(max_lo, n_lo)
        max_hi = max(max_hi, n_hi)
    s_lo = (max_lo + P - 1) // P
    s_hi = (max_hi + P - 1) // P
    S = s_lo + s_hi

    in_maps = []
    for c in range(N_CORES):
        sgi = np.zeros((n_blocks, S * P), np.int16)
        rgi = np.zeros((n_blocks, S * P), np.int16)
        dgi = np.zeros((n_blocks, S * P), np.int16)
        doh = np.full((n_blocks, S * P), float(P), np.float32)
        for b in range(n_blocks):
            g = c * n_blocks + b
            e0, e1, n_lo, n_hi = lohi[g]
            base = c * NPC + b * P
            s_g, d_g, r_g = src_s[e0:e1], dst_s[e0:e1], rel_s[e0:e1]
            is_lo = s_g < LO_ROWS
            for sel, off, cap, sub in ((is_lo, 0, s_lo * P, 0),
                                       (~is_lo, s_lo * P, s_hi * P, LO_ROWS)):
                ss, dd, rr = s_g[sel], d_g[sel], r_g[sel]
                o2 = np.argsort(ss, kind="stable")
                ss, dd, rr = ss[o2], dd[o2], rr[o2]
                n = len(ss)
                assert n <= cap
                sgi[b, off:off + n] = ss - sub
                rgi[b, off:off + n] = rr
                dgi[b, off:off + n] = dd - base
                doh[b, off:off + n] = (dd - base).astype(np.float32)
                if n < cap:  # pad with a repeat of a real edge (or zeros)
                    if n > 0:
                        sgi[b, off + n:off + cap] = ss[0] - sub
                        rgi[b, off + n:off + cap] = rr[0]
                        dgi[b, off + n:off + cap] = dd[0] - base
                    # doh stays 128 -> zero contribution
        # to device layouts
        sgi_l = np.stack([_idx_to_gather_layout(sgi[b]) for b in range(n_blocks)])
        rgi_l = np.stack([_idx_to_gather_layout(rgi[b]) for b in range(n_blocks)])
        dgi_l = np.stack([_idx_to_gather_layout(dgi[b]) for b in range(n_blocks)])
        doh_l = np.stack([doh[b].reshape(S, P).T for b in range(n_blocks)])
        iota = np.broadcast_to(np.arange(P, dtype=np.float32), (P, P)).copy()
        in_maps.append({
            "ent": np.ascontiguousarray(ent_emb, np.float32),
            "ent_loc": np.ascontiguousarray(
                ent_emb[c * NPC:(c + 1) * NPC], np.float32),
            "relt": np.ascontiguousarray(rel_emb, np.float32),
            "w": np.ascontiguousarray(neigh_w, np.float32),
            "iota": iota,
            "src_gi": np.ascontiguousarray(sgi_l.transpose(1, 0, 2)),
            "rel_gi": np.ascontiguousarray(rgi_l.transpose(1, 0, 2)),
            "dst_gi": np.ascontiguousarray(dgi_l.transpose(1, 0, 2)),
            "dst_oh": np.ascontiguousarray(
                doh_l.transpose(1, 0, 2).astype(np.float32)),
        })
    return in_maps, (NPC, n_blocks, s_lo, s_hi, N_ENT, N_REL, LO_ROWS)


LAST_RESULT = None


def _install_ntff_hook():
    """Provide the antenv.axon_hooks module the container's stub lacks, so
    run_bass_kernel_spmd(trace=True) can capture NTFF profiles via libaxon."""
    import sys
    import types
    if "antenv.axon_hooks" in sys.modules:
        return
    mod = types.ModuleType("antenv.axon_hooks")
    hook = [None]
    mod.set_axon_ntff_profile_hook = lambda h: hook.__setitem__(0, h)
    mod.get_axon_ntff_profile_hook = lambda: hook[0]
    sys.modules["antenv.axon_hooks"] = mod
    import antenv
    antenv.axon_hooks = mod
    try:
        from trn_agent_boot.trn_boot import _ntff_profile_via_ctypes
        h = _ntff_profile_via_ctypes("/opt/axon/libaxon_pjrt.so")
        if h is not None:
            mod.set_axon_ntff_profile_hook(lambda *a, **k: h(*a, **k))
    except Exception as e:  # degrade to no-trace
        print("ntff hook install failed:", e)


def kernel(ent_emb, rel_emb, neigh_w, src, dst, rel_id, _trace=False):
    global LAST_RESULT
    from concourse.bass_utils import run_bass_kernel_spmd
    if _trace:
        _install_ntff_hook()

    in_maps, key = _prep_inputs(ent_emb, rel_emb, neigh_w, src, dst, rel_id)
    if key not in _cache:
        _cache[key] = _build_program(*key)
    nc = _cache[key]
    res = run_bass_kernel_spmd(nc, in_maps, list(range(N_CORES)),
                               trace=_trace)
    LAST_RESULT = res
    return np.concatenate([r["out"] for r in res.results], axis=0)



# revision 3
# speedup vs baseline: 1.3090x; 1.3090x over previous
"""GNN message-passing (CompGCN-style edge-softmax) Trainium2 kernel.

Contract: kernel(**inputs) takes FULL unsharded inputs (ent_emb [50000,128] f32,
rel_emb [1000,128] f32, neigh_w [128,128] f32, src/dst/rel_id [600000] int) and
returns the FULL [50000,128] f32 output of:

    comp  = ent_emb[src] * rel_emb[rel_id]
    score = sum(comp * ent_emb[dst], -1)
    alpha = segment_softmax(score, dst)          # grouped by dst
    neigh = segment_sum(comp * alpha[:,None], dst)
    out   = tanh(neigh @ neigh_w)

Sharding: edges are sharded across the 8 cores BY DST RANGE (core c owns dst in
[c*6250,(c+1)*6250)), so segment max/sum are purely core-local and no
collective is needed; ent_emb/rel_emb/neigh_w are replicated.

The SWDGE gather ucode costs ~8 ns per gathered row (GPSIMD-serialized), so
the kernel gathers only TWO rows per edge (ent[src], rel[rel_id]); the old
per-edge ent[dst] gather is replaced by TensorE: the block's 128 dst rows are
loaded with one contiguous DMA (E_blk) and expanded per-edge as
    dstrows[e,h] = sum_j OHT[j,e] * E_blk[j,h]
where OHT is the transposed dst one-hot built on DVE from a host-shipped
row-replicated dst-index map (dohT).  score=sum(comp*dstrows) then proceeds as
before: es=exp(score) (segment-max subtraction skipped: |score| <~ 60 << 88),
W[e,j] = es_e * (dst_e == j) on DVE, and
    accT[h,j] += comp_c.T @ W_c      (TensorE, PSUM)
    den[j]    += W_c.T @ ones        (TensorE, PSUM)
then out_block = tanh((accT.T @ neigh_w) / den) and a contiguous DMA out.

Each block uses EXACT per-block chunk counts (s_lo_b/s_hi_b maxed across the
8 cores so one SPMD program serves all), instead of the global max.
dma_gather indices are int16, so ent_emb (50000 rows) src-gathers are split
into a lo (rows < 32768) and hi (rows >= 32768) gather per block, with edges
sorted by src inside each section.  Padded slots repeat a real edge but carry
dst_oh = 128 which matches no iota value, so they contribute zero to den/acc.
"""

import numpy as np

N_ENT = 50000
N_REL = 1000
N_EDGES = 600000
H = 128
P = 128
N_CORES = 8
NPC = N_ENT // N_CORES          # nodes per core
LO_ROWS = 32768                 # int16 gather split point

_cache = {}


def _build_program(npc, n_ent, n_rel, lo_rows, s_los, s_his):
    """Build the SPMD Bass/Tile program. s_los/s_his: per-block chunk counts."""
    import concourse.bacc as bacc
    import concourse.mybir as mybir
    import concourse.tile as tile

    f32 = mybir.dt.float32
    i16 = mybir.dt.int16
    n_blocks = len(s_los)
    s_tot = [a + b for a, b in zip(s_los, s_his)]
    S_max = max(s_tot)
    W_chunks = sum(s_tot)              # total chunks across blocks
    W_slots = W_chunks * P             # total edge slots across blocks

    nc = bacc.Bacc("TRN2", target_bir_lowering=False, debug=False,
                   num_devices=N_CORES)

    ent = nc.dram_tensor("ent", [n_ent, H], f32, kind="ExternalInput")
    ent_loc = nc.dram_tensor("ent_loc", [npc, H], f32, kind="ExternalInput")
    relt = nc.dram_tensor("relt", [n_rel, H], f32, kind="ExternalInput")
    w_in = nc.dram_tensor("w", [H, H], f32, kind="ExternalInput")
    iota_in = nc.dram_tensor("iota", [P, P], f32, kind="ExternalInput")
    ioc_in = nc.dram_tensor("iota_col", [P, 1], f32, kind="ExternalInput")
    sgi_in = nc.dram_tensor("src_gi", [P, W_chunks * 8], i16,
                            kind="ExternalInput")
    rgi_in = nc.dram_tensor("rel_gi", [P, W_chunks * 8], i16,
                            kind="ExternalInput")
    doh_in = nc.dram_tensor("dst_oh", [P, W_chunks], f32,
                            kind="ExternalInput")
    dohT_in = nc.dram_tensor("dst_ohT", [P, W_slots], f32,
                             kind="ExternalInput")
    out = nc.dram_tensor("out", [npc, H], f32, kind="ExternalOutput")

    import concourse.bass as bass

    def bc(ap, dims):
        # stride-0 broadcast dims: dims is the final [step,count] list
        return bass.AP(ap.tensor, ap.offset, dims)

    with tile.TileContext(nc) as tc:
        with (
            tc.tile_pool(name="const", bufs=1) as constp,
            tc.tile_pool(name="idx", bufs=1) as idxp,
            tc.tile_pool(name="data", bufs=2) as datap,
            tc.tile_pool(name="small", bufs=2) as smallp,
            tc.tile_pool(name="psum", bufs=1, space="PSUM") as psump,
            tc.tile_pool(name="psum1", bufs=1, space="PSUM") as psum1p,
            tc.tile_pool(name="psumb", bufs=1, space="PSUM") as psumbp,
        ):
            iota_t = constp.tile([P, P], f32)
            nc.sync.dma_start(iota_t[:], iota_in[:])
            ioc_t = constp.tile([P, 1], f32)
            nc.sync.dma_start(ioc_t[:], ioc_in[:])
            w_t = constp.tile([H, H], f32)
            nc.sync.dma_start(w_t[:], w_in[:])
            ones_t = constp.tile([P, 1], f32)
            nc.vector.memset(ones_t[:], 1.0)

            sgi_t = idxp.tile([P, W_chunks * 8], i16)
            nc.sync.dma_start(sgi_t[:], sgi_in[:])
            rgi_t = idxp.tile([P, W_chunks * 8], i16)
            nc.sync.dma_start(rgi_t[:], rgi_in[:])
            doh_t = idxp.tile([P, W_chunks], f32)
            nc.sync.dma_start(doh_t[:], doh_in[:])

            coff = 0   # running chunk offset
            for b in range(n_blocks):
                base = b * P
                nodes_b = min(P, npc - base)
                s_lo, s_hi = s_los[b], s_his[b]
                S = s_lo + s_hi
                ns = S * P

                src_rows = datap.tile([P, S_max, H], f32, tag="src")
                rel_rows = datap.tile([P, S_max, H], f32, tag="rel")
                w_oh = datap.tile([P, S_max, H], f32, tag="W")
                oht_t = datap.tile([P, S_max * P], f32, tag="OHT")
                dohT_t = datap.tile([P, S_max * P], f32, tag="dohT")
                eblk_t = datap.tile([P, H], f32, tag="eblk")

                # --- DMAs / gathers for this block ---
                if s_lo > 0:
                    nc.gpsimd.dma_gather(
                        src_rows[:, 0:s_lo, :], ent[0:lo_rows, :],
                        sgi_t[:, coff * 8:(coff + s_lo) * 8],
                        s_lo * P, s_lo * P, H, single_packet=False)
                if s_hi > 0:
                    nc.gpsimd.dma_gather(
                        src_rows[:, s_lo:S, :], ent[lo_rows:n_ent, :],
                        sgi_t[:, (coff + s_lo) * 8:(coff + S) * 8],
                        s_hi * P, s_hi * P, H, single_packet=False)
                nc.gpsimd.dma_gather(
                    rel_rows[:, 0:S, :], relt[:, :],
                    rgi_t[:, coff * 8:(coff + S) * 8],
                    ns, ns, H, single_packet=False)
                nc.sync.dma_start(dohT_t[:, 0:ns],
                                  dohT_in[:, coff * P:coff * P + ns])
                if nodes_b < P:
                    # stale rows beyond nodes_b must be finite (0*NaN=NaN)
                    nc.vector.memset(eblk_t[:], 0.0)
                nc.sync.dma_start(eblk_t[:nodes_b, :],
                                  ent_loc[base:base + nodes_b, :])

                # --- dst one-hot transposed: OHT[j, e] = (dohT[j,e] == j) ---
                ioc_ap = ioc_t[:]
                ioc_b = bc(ioc_ap, [ioc_ap.ap[0], [0, ns]])
                nc.vector.tensor_tensor(
                    out=oht_t[:, 0:ns], in0=dohT_t[:, 0:ns], in1=ioc_b,
                    op=mybir.AluOpType.is_equal)

                # --- dstrows[e, h] = sum_j OHT[j, e] * E_blk[j, h] (PE) ---
                drows_ps = psumbp.tile([P, S_max, H], f32, tag="drows")
                for c in range(S):
                    nc.tensor.matmul(
                        drows_ps[:, c, :],
                        lhsT=oht_t[:, c * P:(c + 1) * P],
                        rhs=eblk_t[:], start=True, stop=True)

                # comp = ent[src] * rel[rel_id]   (in-place over src_rows)
                nc.vector.tensor_tensor(
                    out=src_rows[:, 0:S, :], in0=src_rows[:, 0:S, :],
                    in1=rel_rows[:, 0:S, :], op=mybir.AluOpType.mult)
                # prod = comp * dstrows          (into rel_rows as scratch)
                nc.vector.tensor_tensor(
                    out=rel_rows[:, 0:S, :], in0=src_rows[:, 0:S, :],
                    in1=drows_ps[:, 0:S, :], op=mybir.AluOpType.mult)
                score = smallp.tile([P, S_max], f32, tag="score")
                nc.vector.tensor_reduce(
                    out=score[:, 0:S], in_=rel_rows[:, 0:S, :],
                    axis=mybir.AxisListType.X, op=mybir.AluOpType.add)
                es = smallp.tile([P, S_max], f32, tag="es")
                nc.scalar.activation(
                    out=es[:, 0:S], in_=score[:, 0:S],
                    func=mybir.ActivationFunctionType.Exp)

                # one-hot W[p, c, j] = (dst_oh[p, c] == j) * es[p, c]
                doh_ap = doh_t[:, coff:coff + S]
                doh_b = bc(doh_ap, [doh_ap.ap[0], doh_ap.ap[1], [0, H]])
                iota_ap = iota_t[:]
                iota_b = bc(iota_ap, [iota_ap.ap[0], [0, S], iota_ap.ap[1]])
                nc.vector.tensor_tensor(
                    out=w_oh[:, 0:S, :], in0=doh_b, in1=iota_b,
                    op=mybir.AluOpType.is_equal)
                es_ap = es[:, 0:S]
                es_b = bc(es_ap, [es_ap.ap[0], es_ap.ap[1], [0, H]])
                nc.vector.tensor_tensor(
                    out=w_oh[:, 0:S, :], in0=w_oh[:, 0:S, :], in1=es_b,
                    op=mybir.AluOpType.mult)

                # accT[h, j] = sum_c comp_c.T @ W_c
                acct_ps = psump.tile([P, P], f32, tag="accT")
                for c in range(S):
                    nc.tensor.matmul(
                        acct_ps[:], lhsT=src_rows[:, c, :], rhs=w_oh[:, c, :],
                        start=(c == 0), stop=(c == S - 1))
                # den[j] = sum_c W_c.T @ ones
                den_ps = psum1p.tile([P, 1], f32, tag="den")
                for c in range(S):
                    nc.tensor.matmul(
                        den_ps[:], lhsT=w_oh[:, c, :], rhs=ones_t[:],
                        start=(c == 0), stop=(c == S - 1))

                acct_sb = smallp.tile([P, P], f32, tag="acct_sb")
                nc.scalar.copy(acct_sb[:], acct_ps[:])
                den_sb = smallp.tile([P, 1], f32, tag="den_sb")
                nc.vector.tensor_scalar_max(den_sb[:], den_ps[:], 1e-30)
                rden = smallp.tile([P, 1], f32, tag="rden")
                nc.vector.reciprocal(rden[:], den_sb[:])

                out_ps = psump.tile([P, H], f32, tag="out_ps")
                nc.tensor.matmul(out_ps[:], lhsT=acct_sb[:], rhs=w_t[:],
                                 start=True, stop=True)
                out_sb = smallp.tile([P, H], f32, tag="out_sb")
                nc.scalar.activation(
                    out=out_sb[:], in_=out_ps[:],
                    func=mybir.ActivationFunctionType.Tanh, scale=rden[:])
                nc.sync.dma_start(out[base:base + nodes_b, :],
                                  out_sb[:nodes_b, :])
                coff += S

    nc.compile()
    return nc


def _idx_to_gather_layout(arr):
    """[n*128] int16 gather-position-ordered indices -> [128, n*8] tile."""
    a = arr.reshape(-1, 16).T.astype(np.int16)      # [16, n*8]
    return np.tile(a, (8, 1))                        # [128, n*8]


def _prep_inputs(ent_emb, rel_emb, neigh_w, src, dst, rel_id):
    """Partition edges by dst core/block, build per-core gather index arrays.

    Per-block chunk counts (s_lo_b, s_hi_b) are exact per block, maxed across
    cores so one SPMD program serves all 8 cores.
    """
    src = np.asarray(src).astype(np.int64)
    dst = np.asarray(dst).astype(np.int64)
    rel_id = np.asarray(rel_id).astype(np.int64)
    n_blocks = (NPC + P - 1) // P

    order = np.argsort(dst, kind="stable")
    src_s, dst_s, rel_s = src[order], dst[order], rel_id[order]
    g_s = (dst_s // NPC) * n_blocks + (dst_s % NPC) // P
    n_gblocks = N_CORES * n_blocks
    bounds = np.searchsorted(g_s, np.arange(n_gblocks + 1))

    # per-(core,block) lo/hi counts -> per-block max across cores
    lohi = []
    for g in range(n_gblocks):
        e0, e1 = bounds[g], bounds[g + 1]
        s_g = src_s[e0:e1]
        n_lo = int((s_g < LO_ROWS).sum())
        n_hi = int(e1 - e0 - n_lo)
        lohi.append((e0, e1, n_lo, n_hi))
    s_los, s_his = [], []
    for b in range(n_blocks):
        ml = max(max(lohi[c * n_blocks + b][2] for c in range(N_CORES)), 1)
        mh = max(lohi[c * n_blocks + b][3] for c in range(N_CORES))
        s_los.append((ml + P - 1) // P)
        s_his.append((mh + P - 1) // P)
    s_tot = [a + b for a, b in zip(s_los, s_his)]
    W_chunks = sum(s_tot)

    iota = np.broadcast_to(np.arange(P, dtype=np.float32), (P, P)).copy()
    iota_col = np.arange(P, dtype=np.float32).reshape(P, 1).copy()

    in_maps = []
    for c in range(N_CORES):
        sgi = np.zeros((W_chunks * P,), np.int16)
        rgi = np.zeros((W_chunks * P,), np.int16)
        doh = np.full((W_chunks * P,), float(P), np.float32)
        coff = 0
        for b in range(n_blocks):
            g = c * n_blocks + b
            e0, e1, n_lo, n_hi = lohi[g]
            s_lo, s_hi = s_los[b], s_his[b]
            base = c * NPC + b * P
            s_g, d_g, r_g = src_s[e0:e1], dst_s[e0:e1], rel_s[e0:e1]
            is_lo = s_g < LO_ROWS
            o0 = coff * P
            for sel, off, cap, sub in ((is_lo, 0, s_lo * P, 0),
                                       (~is_lo, s_lo * P, s_hi * P, LO_ROWS)):
                ss, dd, rr = s_g[sel], d_g[sel], r_g[sel]
                o2 = np.argsort(ss, kind="stable")
                ss, dd, rr = ss[o2], dd[o2], rr[o2]
                n = len(ss)
                assert n <= cap
                sgi[o0 + off:o0 + off + n] = ss - sub
                rgi[o0 + off:o0 + off + n] = rr
                doh[o0 + off:o0 + off + n] = (dd - base).astype(np.float32)
                if n < cap and n > 0:  # pad with a repeat of a real edge
                    sgi[o0 + off + n:o0 + off + cap] = ss[0] - sub
                    rgi[o0 + off + n:o0 + off + cap] = rr[0]
                    # doh stays 128 -> zero contribution
            coff += s_lo + s_hi

        # device layouts: idx 16-wrap per block section, doh per-chunk columns
        sgi_cols, rgi_cols, doh_cols = [], [], []
        coff = 0
        for b in range(n_blocks):
            s_lo, s_hi, S = s_los[b], s_his[b], s_tot[b]
            o0 = coff * P
            lo_a = _idx_to_gather_layout(sgi[o0:o0 + s_lo * P])
            hi_a = (_idx_to_gather_layout(sgi[o0 + s_lo * P:o0 + S * P])
                    if s_hi > 0 else np.zeros((P, 0), np.int16))
            sgi_cols.append(np.concatenate([lo_a, hi_a], axis=1))
            rgi_cols.append(_idx_to_gather_layout(rgi[o0:o0 + S * P]))
            doh_cols.append(doh[o0:o0 + S * P].reshape(S, P).T)
            coff += S
        sgi_l = np.concatenate(sgi_cols, axis=1)
        rgi_l = np.concatenate(rgi_cols, axis=1)
        doh_l = np.concatenate(doh_cols, axis=1)
        dohT = np.broadcast_to(doh[None, :], (P, W_chunks * P))

        in_maps.append({
            "ent": np.ascontiguousarray(ent_emb, np.float32),
            "ent_loc": np.ascontiguousarray(
                ent_emb[c * NPC:(c + 1) * NPC], np.float32),
            "relt": np.ascontiguousarray(rel_emb, np.float32),
            "w": np.ascontiguousarray(neigh_w, np.float32),
            "iota": iota,
            "iota_col": iota_col,
            "src_gi": np.ascontiguousarray(sgi_l),
            "rel_gi": np.ascontiguousarray(rgi_l),
            "dst_oh": np.ascontiguousarray(doh_l),
            "dst_ohT": np.ascontiguousarray(dohT),
        })
    return in_maps, (NPC, N_ENT, N_REL, LO_ROWS, tuple(s_los), tuple(s_his))


LAST_RESULT = None


def _install_ntff_hook():
    """Provide the antenv.axon_hooks module the container's stub lacks, so
    run_bass_kernel_spmd(trace=True) can capture NTFF profiles via libaxon."""
    import sys
    import types
    if "antenv.axon_hooks" in sys.modules:
        return
    mod = types.ModuleType("antenv.axon_hooks")
    hook = [None]
    mod.set_axon_ntff_profile_hook = lambda h: hook.__setitem__(0, h)
    mod.get_axon_ntff_profile_hook = lambda: hook[0]
    sys.modules["antenv.axon_hooks"] = mod
    import antenv
    antenv.axon_hooks = mod
    try:
        from trn_agent_boot.trn_boot import _ntff_profile_via_ctypes
        h = _ntff_profile_via_ctypes("/opt/axon/libaxon_pjrt.so")
        if h is not None:
            mod.set_axon_ntff_profile_hook(lambda *a, **k: h(*a, **k))
    except Exception as e:  # degrade to no-trace
        print("ntff hook install failed:", e)


def kernel(ent_emb, rel_emb, neigh_w, src, dst, rel_id, _trace=False):
    global LAST_RESULT
    from concourse.bass_utils import run_bass_kernel_spmd
    if _trace:
        _install_ntff_hook()

    in_maps, key = _prep_inputs(ent_emb, rel_emb, neigh_w, src, dst, rel_id)
    if key not in _cache:
        _cache[key] = _build_program(key[0], key[1], key[2], key[3],
                                     list(key[4]), list(key[5]))
    nc = _cache[key]
    res = run_bass_kernel_spmd(nc, in_maps, list(range(N_CORES)),
                               trace=_trace)
    LAST_RESULT = res
    return np.concatenate([r["out"] for r in res.results], axis=0)


# revision 5
# speedup vs baseline: 1.4664x; 1.1202x over previous
"""GNN message-passing (CompGCN-style edge-softmax) Trainium2 kernel.

Contract: kernel(**inputs) takes FULL unsharded inputs (ent_emb [50000,128] f32,
rel_emb [1000,128] f32, neigh_w [128,128] f32, src/dst/rel_id [600000] int) and
returns the FULL [50000,128] f32 output of:

    comp  = ent_emb[src] * rel_emb[rel_id]
    score = sum(comp * ent_emb[dst], -1)
    alpha = segment_softmax(score, dst)          # grouped by dst
    neigh = segment_sum(comp * alpha[:,None], dst)
    out   = tanh(neigh @ neigh_w)

Sharding: edges are sharded across the 8 cores BY DST RANGE (core c owns dst in
[c*6250,(c+1)*6250)), so segment max/sum are purely core-local and no
collective is needed; ent_emb/rel_emb/neigh_w are replicated.

The SWDGE gather ucode costs ~8 ns per gathered row (GPSIMD-serialized), so
the kernel gathers only TWO rows per edge (ent[src], rel[rel_id]); the old
per-edge ent[dst] gather is replaced by TensorE: the block's 128 dst rows are
loaded with one contiguous DMA (E_blk) and expanded per-edge as
    dstrows[e,h] = sum_j OHT[j,e] * E_blk[j,h]
where OHT is the transposed dst one-hot built on DVE from a host-shipped
row-replicated dst-index map (dohT).  score=sum(comp*dstrows) then proceeds as
before: es=exp(score) (segment-max subtraction skipped: |score| <~ 60 << 88),
W[e,j] = es_e * (dst_e == j) on DVE, and
    accT[h,j] += comp_c.T @ W_c      (TensorE, PSUM)
    den[j]    += W_c.T @ ones        (TensorE, PSUM)
then out_block = tanh((accT.T @ neigh_w) / den) and a contiguous DMA out.

Each block uses EXACT per-block chunk counts (s_lo_b/s_hi_b maxed across the
8 cores so one SPMD program serves all), instead of the global max.
dma_gather indices are int16, so ent_emb (50000 rows) src-gathers are split
into a lo (rows < 32768) and hi (rows >= 32768) gather per block, with edges
sorted by src inside each section.  Padded slots repeat a real edge but carry
dst_oh = 128 which matches no iota value, so they contribute zero to den/acc.
"""

import numpy as np

N_ENT = 50000
N_REL = 1000
N_EDGES = 600000
H = 128
P = 128
N_CORES = 8
NPC = N_ENT // N_CORES          # nodes per core
LO_ROWS = 32768                 # int16 gather split point

_cache = {}


def _build_program(npc, n_ent, n_rel, lo_rows, s_los, s_his):
    """Build the SPMD Bass/Tile program. s_los/s_his: per-block chunk counts."""
    import concourse.bacc as bacc
    import concourse.mybir as mybir
    import concourse.tile as tile

    f32 = mybir.dt.float32
    i16 = mybir.dt.int16
    n_blocks = len(s_los)
    s_tot = [a + b for a, b in zip(s_los, s_his)]
    S_max = max(s_tot)
    W_chunks = sum(s_tot)              # total chunks across blocks
    W_slots = W_chunks * P             # total edge slots across blocks

    nc = bacc.Bacc("TRN2", target_bir_lowering=False, debug=False,
                   num_devices=N_CORES)

    ent = nc.dram_tensor("ent", [n_ent, H], f32, kind="ExternalInput")
    ent_loc = nc.dram_tensor("ent_loc", [npc, H], f32, kind="ExternalInput")
    relt = nc.dram_tensor("relt", [n_rel, H], f32, kind="ExternalInput")
    w_in = nc.dram_tensor("w", [H, H], f32, kind="ExternalInput")
    iota_in = nc.dram_tensor("iota", [P, P], f32, kind="ExternalInput")
    ioc_in = nc.dram_tensor("iota_col", [P, 1], f32, kind="ExternalInput")
    sgi_in = nc.dram_tensor("src_gi", [P, W_chunks * 8], i16,
                            kind="ExternalInput")
    rgi_in = nc.dram_tensor("rel_gi", [P, W_chunks * 8], i16,
                            kind="ExternalInput")
    doh_in = nc.dram_tensor("dst_oh", [P, W_chunks], f32,
                            kind="ExternalInput")
    dohT_in = nc.dram_tensor("dst_ohT", [P, W_slots], f32,
                             kind="ExternalInput")
    out = nc.dram_tensor("out", [npc, H], f32, kind="ExternalOutput")

    import concourse.bass as bass

    def bc(ap, dims):
        # stride-0 broadcast dims: dims is the final [step,count] list
        return bass.AP(ap.tensor, ap.offset, dims)

    with tile.TileContext(nc) as tc:
        with (
            tc.tile_pool(name="const", bufs=1) as constp,
            tc.tile_pool(name="idx", bufs=1) as idxp,
            tc.tile_pool(name="data", bufs=3) as datap,
            tc.tile_pool(name="small", bufs=2) as smallp,
            tc.tile_pool(name="psum", bufs=1, space="PSUM") as psump,
            tc.tile_pool(name="psum1", bufs=1, space="PSUM") as psum1p,
            tc.tile_pool(name="psumb", bufs=1, space="PSUM") as psumbp,
        ):
            iota_t = constp.tile([P, P], f32)
            nc.sync.dma_start(iota_t[:], iota_in[:])
            ioc_t = constp.tile([P, 1], f32)
            nc.sync.dma_start(ioc_t[:], ioc_in[:])
            w_t = constp.tile([H, H], f32)
            nc.sync.dma_start(w_t[:], w_in[:])
            ones_t = constp.tile([P, 1], f32)
            nc.vector.memset(ones_t[:], 1.0)

            sgi_t = idxp.tile([P, W_chunks * 8], i16)
            nc.sync.dma_start(sgi_t[:], sgi_in[:])
            rgi_t = idxp.tile([P, W_chunks * 8], i16)
            nc.sync.dma_start(rgi_t[:], rgi_in[:])
            doh_t = idxp.tile([P, W_chunks], f32)
            nc.sync.dma_start(doh_t[:], doh_in[:])

            coff = 0   # running chunk offset
            for b in range(n_blocks):
                base = b * P
                nodes_b = min(P, npc - base)
                s_lo, s_hi = s_los[b], s_his[b]
                S = s_lo + s_hi
                ns = S * P

                src_rows = datap.tile([P, S_max, H], f32, tag="src")
                rel_rows = datap.tile([P, S_max, H], f32, tag="rel")
                w_oh = datap.tile([P, S_max, H], f32, tag="W")
                oht_t = datap.tile([P, S_max * P], f32, tag="OHT")
                dohT_t = datap.tile([P, S_max * P], f32, tag="dohT")
                eblk_t = datap.tile([P, H], f32, tag="eblk")

                # --- DMAs / gathers for this block ---
                if s_lo > 0:
                    nc.gpsimd.dma_gather(
                        src_rows[:, 0:s_lo, :], ent[0:lo_rows, :],
                        sgi_t[:, coff * 8:(coff + s_lo) * 8],
                        s_lo * P, s_lo * P, H, single_packet=False)
                if s_hi > 0:
                    nc.gpsimd.dma_gather(
                        src_rows[:, s_lo:S, :], ent[lo_rows:n_ent, :],
                        sgi_t[:, (coff + s_lo) * 8:(coff + S) * 8],
                        s_hi * P, s_hi * P, H, single_packet=False)
                nc.gpsimd.dma_gather(
                    rel_rows[:, 0:S, :], relt[:, :],
                    rgi_t[:, coff * 8:(coff + S) * 8],
                    ns, ns, H, single_packet=False)
                nc.sync.dma_start(dohT_t[:, 0:ns],
                                  dohT_in[:, coff * P:coff * P + ns])
                if nodes_b < P:
                    # stale rows beyond nodes_b must be finite (0*NaN=NaN)
                    nc.vector.memset(eblk_t[:], 0.0)
                nc.sync.dma_start(eblk_t[:nodes_b, :],
                                  ent_loc[base:base + nodes_b, :])

                # --- dst one-hot transposed: OHT[j, e] = (dohT[j,e] == j) ---
                ioc_ap = ioc_t[:]
                ioc_b = bc(ioc_ap, [ioc_ap.ap[0], [0, ns]])
                nc.vector.tensor_tensor(
                    out=oht_t[:, 0:ns], in0=dohT_t[:, 0:ns], in1=ioc_b,
                    op=mybir.AluOpType.is_equal)

                # --- dstrows[e, h] = sum_j OHT[j, e] * E_blk[j, h] (PE) ---
                drows_ps = psumbp.tile([P, S_max, H], f32, tag="drows")
                for c in range(S):
                    nc.tensor.matmul(
                        drows_ps[:, c, :],
                        lhsT=oht_t[:, c * P:(c + 1) * P],
                        rhs=eblk_t[:], start=True, stop=True)

                # comp = ent[src] * rel[rel_id]   (in-place over src_rows)
                nc.vector.tensor_tensor(
                    out=src_rows[:, 0:S, :], in0=src_rows[:, 0:S, :],
                    in1=rel_rows[:, 0:S, :], op=mybir.AluOpType.mult)
                # prod = comp * dstrows          (into rel_rows as scratch)
                nc.vector.tensor_tensor(
                    out=rel_rows[:, 0:S, :], in0=src_rows[:, 0:S, :],
                    in1=drows_ps[:, 0:S, :], op=mybir.AluOpType.mult)
                score = smallp.tile([P, S_max], f32, tag="score")
                nc.vector.tensor_reduce(
                    out=score[:, 0:S], in_=rel_rows[:, 0:S, :],
                    axis=mybir.AxisListType.X, op=mybir.AluOpType.add)
                es = smallp.tile([P, S_max], f32, tag="es")
                nc.scalar.activation(
                    out=es[:, 0:S], in_=score[:, 0:S],
                    func=mybir.ActivationFunctionType.Exp)

                # one-hot W[p, c, j] = (dst_oh[p, c] == j) * es[p, c]
                doh_ap = doh_t[:, coff:coff + S]
                doh_b = bc(doh_ap, [doh_ap.ap[0], doh_ap.ap[1], [0, H]])
                iota_ap = iota_t[:]
                iota_b = bc(iota_ap, [iota_ap.ap[0], [0, S], iota_ap.ap[1]])
                nc.vector.tensor_tensor(
                    out=w_oh[:, 0:S, :], in0=doh_b, in1=iota_b,
                    op=mybir.AluOpType.is_equal)
                es_ap = es[:, 0:S]
                es_b = bc(es_ap, [es_ap.ap[0], es_ap.ap[1], [0, H]])
                nc.vector.tensor_tensor(
                    out=w_oh[:, 0:S, :], in0=w_oh[:, 0:S, :], in1=es_b,
                    op=mybir.AluOpType.mult)

                # accT[h, j] = sum_c comp_c.T @ W_c
                acct_ps = psump.tile([P, P], f32, tag="accT")
                for c in range(S):
                    nc.tensor.matmul(
                        acct_ps[:], lhsT=src_rows[:, c, :], rhs=w_oh[:, c, :],
                        start=(c == 0), stop=(c == S - 1))
                # den[j] = sum_c W_c.T @ ones
                den_ps = psum1p.tile([P, 1], f32, tag="den")
                for c in range(S):
                    nc.tensor.matmul(
                        den_ps[:], lhsT=w_oh[:, c, :], rhs=ones_t[:],
                        start=(c == 0), stop=(c == S - 1))

                acct_sb = smallp.tile([P, P], f32, tag="acct_sb")
                nc.scalar.copy(acct_sb[:], acct_ps[:])
                den_sb = smallp.tile([P, 1], f32, tag="den_sb")
                nc.vector.tensor_scalar_max(den_sb[:], den_ps[:], 1e-30)
                rden = smallp.tile([P, 1], f32, tag="rden")
                nc.vector.reciprocal(rden[:], den_sb[:])

                out_ps = psump.tile([P, H], f32, tag="out_ps")
                nc.tensor.matmul(out_ps[:], lhsT=acct_sb[:], rhs=w_t[:],
                                 start=True, stop=True)
                out_sb = smallp.tile([P, H], f32, tag="out_sb")
                nc.scalar.activation(
                    out=out_sb[:], in_=out_ps[:],
                    func=mybir.ActivationFunctionType.Tanh, scale=rden[:])
                nc.sync.dma_start(out[base:base + nodes_b, :],
                                  out_sb[:nodes_b, :])
                coff += S

    nc.compile()
    return nc


def _idx_to_gather_layout(arr):
    """[n*128] int16 gather-position-ordered indices -> [128, n*8] tile."""
    a = arr.reshape(-1, 16).T.astype(np.int16)      # [16, n*8]
    return np.tile(a, (8, 1))                        # [128, n*8]


def _prep_inputs(ent_emb, rel_emb, neigh_w, src, dst, rel_id):
    """Partition edges by dst core/block, build per-core gather index arrays.

    Per-block chunk counts (s_lo_b, s_hi_b) are exact per block, maxed across
    cores so one SPMD program serves all 8 cores.
    """
    src = np.asarray(src).astype(np.int64)
    dst = np.asarray(dst).astype(np.int64)
    rel_id = np.asarray(rel_id).astype(np.int64)
    n_blocks = (NPC + P - 1) // P

    order = np.argsort(dst, kind="stable")
    src_s, dst_s, rel_s = src[order], dst[order], rel_id[order]
    g_s = (dst_s // NPC) * n_blocks + (dst_s % NPC) // P
    n_gblocks = N_CORES * n_blocks
    bounds = np.searchsorted(g_s, np.arange(n_gblocks + 1))

    # per-(core,block) lo/hi counts -> per-block max across cores
    lohi = []
    for g in range(n_gblocks):
        e0, e1 = bounds[g], bounds[g + 1]
        s_g = src_s[e0:e1]
        n_lo = int((s_g < LO_ROWS).sum())
        n_hi = int(e1 - e0 - n_lo)
        lohi.append((e0, e1, n_lo, n_hi))
    s_los, s_his = [], []
    for b in range(n_blocks):
        ml = max(max(lohi[c * n_blocks + b][2] for c in range(N_CORES)), 1)
        mh = max(lohi[c * n_blocks + b][3] for c in range(N_CORES))
        s_los.append((ml + P - 1) // P)
        s_his.append((mh + P - 1) // P)
    s_tot = [a + b for a, b in zip(s_los, s_his)]
    W_chunks = sum(s_tot)

    iota = np.broadcast_to(np.arange(P, dtype=np.float32), (P, P)).copy()
    iota_col = np.arange(P, dtype=np.float32).reshape(P, 1).copy()

    in_maps = []
    for c in range(N_CORES):
        sgi = np.zeros((W_chunks * P,), np.int16)
        rgi = np.zeros((W_chunks * P,), np.int16)
        doh = np.full((W_chunks * P,), float(P), np.float32)
        coff = 0
        for b in range(n_blocks):
            g = c * n_blocks + b
            e0, e1, n_lo, n_hi = lohi[g]
            s_lo, s_hi = s_los[b], s_his[b]
            base = c * NPC + b * P
            s_g, d_g, r_g = src_s[e0:e1], dst_s[e0:e1], rel_s[e0:e1]
            is_lo = s_g < LO_ROWS
            o0 = coff * P
            for sel, off, cap, sub in ((is_lo, 0, s_lo * P, 0),
                                       (~is_lo, s_lo * P, s_hi * P, LO_ROWS)):
                ss, dd, rr = s_g[sel], d_g[sel], r_g[sel]
                o2 = np.argsort(ss, kind="stable")
                ss, dd, rr = ss[o2], dd[o2], rr[o2]
                n = len(ss)
                assert n <= cap
                sgi[o0 + off:o0 + off + n] = ss - sub
                rgi[o0 + off:o0 + off + n] = rr
                doh[o0 + off:o0 + off + n] = (dd - base).astype(np.float32)
                if n < cap and n > 0:  # pad with a repeat of a real edge
                    sgi[o0 + off + n:o0 + off + cap] = ss[0] - sub
                    rgi[o0 + off + n:o0 + off + cap] = rr[0]
                    # doh stays 128 -> zero contribution
            coff += s_lo + s_hi

        # device layouts: idx 16-wrap per block section, doh per-chunk columns
        sgi_cols, rgi_cols, doh_cols = [], [], []
        coff = 0
        for b in range(n_blocks):
            s_lo, s_hi, S = s_los[b], s_his[b], s_tot[b]
            o0 = coff * P
            lo_a = _idx_to_gather_layout(sgi[o0:o0 + s_lo * P])
            hi_a = (_idx_to_gather_layout(sgi[o0 + s_lo * P:o0 + S * P])
                    if s_hi > 0 else np.zeros((P, 0), np.int16))
            sgi_cols.append(np.concatenate([lo_a, hi_a], axis=1))
            rgi_cols.append(_idx_to_gather_layout(rgi[o0:o0 + S * P]))
            doh_cols.append(doh[o0:o0 + S * P].reshape(S, P).T)
            coff += S
        sgi_l = np.concatenate(sgi_cols, axis=1)
        rgi_l = np.concatenate(rgi_cols, axis=1)
        doh_l = np.concatenate(doh_cols, axis=1)
        dohT = np.broadcast_to(doh[None, :], (P, W_chunks * P))

        in_maps.append({
            "ent": np.ascontiguousarray(ent_emb, np.float32),
            "ent_loc": np.ascontiguousarray(
                ent_emb[c * NPC:(c + 1) * NPC], np.float32),
            "relt": np.ascontiguousarray(rel_emb, np.float32),
            "w": np.ascontiguousarray(neigh_w, np.float32),
            "iota": iota,
            "iota_col": iota_col,
            "src_gi": np.ascontiguousarray(sgi_l),
            "rel_gi": np.ascontiguousarray(rgi_l),
            "dst_oh": np.ascontiguousarray(doh_l),
            "dst_ohT": np.ascontiguousarray(dohT),
        })
    return in_maps, (NPC, N_ENT, N_REL, LO_ROWS, tuple(s_los), tuple(s_his))


LAST_RESULT = None


def _install_ntff_hook():
    """Provide the antenv.axon_hooks module the container's stub lacks, so
    run_bass_kernel_spmd(trace=True) can capture NTFF profiles via libaxon."""
    import sys
    import types
    if "antenv.axon_hooks" in sys.modules:
        return
    mod = types.ModuleType("antenv.axon_hooks")
    hook = [None]
    mod.set_axon_ntff_profile_hook = lambda h: hook.__setitem__(0, h)
    mod.get_axon_ntff_profile_hook = lambda: hook[0]
    sys.modules["antenv.axon_hooks"] = mod
    import antenv
    antenv.axon_hooks = mod
    try:
        from trn_agent_boot.trn_boot import _ntff_profile_via_ctypes
        h = _ntff_profile_via_ctypes("/opt/axon/libaxon_pjrt.so")
        if h is not None:
            mod.set_axon_ntff_profile_hook(lambda *a, **k: h(*a, **k))
    except Exception as e:  # degrade to no-trace
        print("ntff hook install failed:", e)


def kernel(ent_emb, rel_emb, neigh_w, src, dst, rel_id, _trace=False):
    global LAST_RESULT
    from concourse.bass_utils import run_bass_kernel_spmd
    if _trace:
        _install_ntff_hook()

    in_maps, key = _prep_inputs(ent_emb, rel_emb, neigh_w, src, dst, rel_id)
    if key not in _cache:
        _cache[key] = _build_program(key[0], key[1], key[2], key[3],
                                     list(key[4]), list(key[5]))
    nc = _cache[key]
    res = run_bass_kernel_spmd(nc, in_maps, list(range(N_CORES)),
                               trace=_trace)
    LAST_RESULT = res
    return np.concatenate([r["out"] for r in res.results], axis=0)


# revision 6
# speedup vs baseline: 2.1194x; 1.4453x over previous
"""GNN message-passing (CompGCN edge-softmax) TRN2 kernel — no rel gather.

Same contract/sharding as kernel.py (edges sharded by dst range, 8 cores).

SWDGE gather costs ~8 ns/row on GPSIMD, so this version gathers ONLY
ent[src] (one row per edge).  The other two per-edge rows come from TensorE:

  dst rows:  dstrows[e,h] = sum_j OHT[j,e]*E_blk[j,h]   (E_blk = block's 128
             local dst rows, contiguous DMA, dual-bf16 hi+lo for exactness)
  rel rows:  relrows[e,h] = sum_r rOHT[r',e]*rel_q[r',h] (rel table SBUF-
             resident in 8 chunks of 128 rows, dual-bf16 hi+lo; edges are
             rel-sorted within each block section with per-(section,q) slot
             counts FIXED across cores, so each 128-slot chunk intersects a
             couple of compile-time-known q ranges -> partition-offset
             matmuls)

score = sum(comp * dstrows) stays fp32-exact (dual-bf16 residual ~1e-4).
Aggregation runs in bf16 (comp cast + W one-hot*es), err ~1e-2 < 2e-2 tol:
    accT[h,j] += comp_c.T @ W_c      (bf16 TensorE, fp32 PSUM)
    den[j]     = Wsum.T @ ones       (Wsum = sum_c W_c on DVE)
    out_block  = tanh((accT.T @ neigh_w)/den)
"""

import numpy as np

N_ENT = 50000
N_REL = 1000
H = 128
P = 128
NQ = 8                      # rel table chunks of 128 rows
N_CORES = 8
NPC = N_ENT // N_CORES
LO_ROWS = 32768

_cache = {}


def _bfsplit(x):
    import ml_dtypes
    hi = x.astype(np.float32).astype(ml_dtypes.bfloat16)
    lo = (x.astype(np.float32) - hi.astype(np.float32)).astype(ml_dtypes.bfloat16)
    return hi, lo


def _build_program(npc, n_ent, lo_rows, s_los, s_his, runs_all, n_par):
    """runs_all[b] = list of (c, q, s0, s1); q%n_par picks the parity
    one-hot tile whose rows are zero outside q's slots."""
    import concourse.bacc as bacc
    import concourse.mybir as mybir
    import concourse.tile as tile

    f32 = mybir.dt.float32
    f16 = mybir.dt.float16
    bf16 = mybir.dt.bfloat16
    i16 = mybir.dt.int16
    n_blocks = len(s_los)
    s_tot = [a + b for a, b in zip(s_los, s_his)]
    S_max = max(s_tot)
    W_chunks = sum(s_tot)
    W_slots = W_chunks * P

    nc = bacc.Bacc("TRN2", target_bir_lowering=False, debug=False,
                   num_devices=N_CORES)

    ent = nc.dram_tensor("ent", [n_ent, H], f32, kind="ExternalInput")
    elh_in = nc.dram_tensor("ent_loc_hi", [npc, H], bf16, kind="ExternalInput")
    ell_in = nc.dram_tensor("ent_loc_lo", [npc, H], bf16, kind="ExternalInput")
    rlh_in = nc.dram_tensor("rel_hi", [P, NQ, H], bf16, kind="ExternalInput")
    rll_in = nc.dram_tensor("rel_lo", [P, NQ, H], bf16, kind="ExternalInput")
    w_in = nc.dram_tensor("w", [H, H], f32, kind="ExternalInput")
    iota_in = nc.dram_tensor("iota", [P, P], f32, kind="ExternalInput")
    ioc_in = nc.dram_tensor("iota_col", [P, 1], f32, kind="ExternalInput")
    ioc16_in = nc.dram_tensor("iota_col16", [P, 1], f16, kind="ExternalInput")
    sgi_in = nc.dram_tensor("src_gi", [P, W_chunks * 8], i16,
                            kind="ExternalInput")
    doh_in = nc.dram_tensor("dst_oh", [P, W_chunks], f32,
                            kind="ExternalInput")
    dohT_in = nc.dram_tensor("dst_ohT", [P, W_slots], f32,
                             kind="ExternalInput")
    ridT_in = [nc.dram_tensor(f"relidT{p}", [P, W_slots], f16,
                              kind="ExternalInput") for p in range(n_par)]
    out = nc.dram_tensor("out", [npc, H], f32, kind="ExternalOutput")

    import concourse.bass as bass

    def bc(ap, dims):
        return bass.AP(ap.tensor, ap.offset, dims)

    with tile.TileContext(nc) as tc:
        with (
            tc.tile_pool(name="const", bufs=1) as constp,
            tc.tile_pool(name="idx", bufs=1) as idxp,
            tc.tile_pool(name="data", bufs=2) as datap,
            tc.tile_pool(name="small", bufs=2) as smallp,
            tc.tile_pool(name="psum", bufs=1, space="PSUM") as psump,
            tc.tile_pool(name="psumr", bufs=2, space="PSUM") as psumrp,
            tc.tile_pool(name="psumb", bufs=1, space="PSUM") as psumbp,
        ):
            iota_t = constp.tile([P, P], f32)
            nc.sync.dma_start(iota_t[:], iota_in[:])
            ioc_t = constp.tile([P, 1], f32)
            nc.sync.dma_start(ioc_t[:], ioc_in[:])
            ioc16_t = constp.tile([P, 1], f16)
            nc.sync.dma_start(ioc16_t[:], ioc16_in[:])
            w_t = constp.tile([H, H], f32)
            nc.sync.dma_start(w_t[:], w_in[:])
            rlh_t = constp.tile([P, NQ, H], bf16)
            nc.sync.dma_start(rlh_t[:], rlh_in[:])
            rll_t = constp.tile([P, NQ, H], bf16)
            nc.sync.dma_start(rll_t[:], rll_in[:])
            ones_bf = constp.tile([P, 1], bf16)
            nc.vector.memset(ones_bf[:], 1.0)

            sgi_t = idxp.tile([P, W_chunks * 8], i16)
            nc.sync.dma_start(sgi_t[:], sgi_in[:])
            doh_t = idxp.tile([P, W_chunks], f32)
            nc.sync.dma_start(doh_t[:], doh_in[:])

            coff = 0
            for b in range(n_blocks):
                base = b * P
                nodes_b = min(P, npc - base)
                s_lo, s_hi = s_los[b], s_his[b]
                S = s_lo + s_hi
                ns = S * P

                src_rows = datap.tile([P, S_max, H], f32, tag="src")
                relrows = datap.tile([P, S_max, H], f32, tag="relrows")
                w_oh = datap.tile([P, S_max, H], f32, tag="W")
                comp_bf = datap.tile([P, S_max, H], bf16, tag="compbf")
                w_bf = datap.tile([P, S_max, H], bf16, tag="Wbf")
                oht_t = datap.tile([P, S_max * P], bf16, tag="OHT")
                roht_t = [datap.tile([P, S_max * P], bf16, tag=f"rOHT{p}",
                                     name=f"roht{p}") for p in range(n_par)]
                dohT_t = datap.tile([P, S_max * P], f32, tag="dohT")
                ridT_t = [datap.tile([P, S_max * P], f16, tag=f"ridT{p}",
                                     name=f"ridt{p}") for p in range(n_par)]
                ehi_t = datap.tile([P, H], bf16, tag="ehi")
                elo_t = datap.tile([P, H], bf16, tag="elo")

                if s_lo > 0:
                    nc.gpsimd.dma_gather(
                        src_rows[:, 0:s_lo, :], ent[0:lo_rows, :],
                        sgi_t[:, coff * 8:(coff + s_lo) * 8],
                        s_lo * P, s_lo * P, H, single_packet=False)
                if s_hi > 0:
                    nc.gpsimd.dma_gather(
                        src_rows[:, s_lo:S, :], ent[lo_rows:n_ent, :],
                        sgi_t[:, (coff + s_lo) * 8:(coff + S) * 8],
                        s_hi * P, s_hi * P, H, single_packet=False)
                nc.sync.dma_start(dohT_t[:, 0:ns],
                                  dohT_in[:, coff * P:coff * P + ns])
                for p in range(n_par):
                    nc.sync.dma_start(ridT_t[p][:, 0:ns],
                                      ridT_in[p][:, coff * P:coff * P + ns])
                if nodes_b < P:
                    nc.vector.memset(ehi_t[:], 0.0)
                    nc.vector.memset(elo_t[:], 0.0)
                nc.sync.dma_start(ehi_t[:nodes_b, :],
                                  elh_in[base:base + nodes_b, :])
                nc.sync.dma_start(elo_t[:nodes_b, :],
                                  ell_in[base:base + nodes_b, :])

                # transposed one-hots (bf16 out)
                ioc_ap = ioc_t[:]
                nc.vector.tensor_tensor(
                    out=oht_t[:, 0:ns], in0=dohT_t[:, 0:ns],
                    in1=bc(ioc_ap, [ioc_ap.ap[0], [0, ns]]),
                    op=mybir.AluOpType.is_equal)
                i16_ap = ioc16_t[:]
                for p in range(n_par):
                    nc.vector.tensor_tensor(
                        out=roht_t[p][:, 0:ns], in0=ridT_t[p][:, 0:ns],
                        in1=bc(i16_ap, [i16_ap.ap[0], [0, ns]]),
                        op=mybir.AluOpType.is_equal)

                # dstrows[e,h] = OHT_c.T @ (E_hi + E_lo)
                drows_ps = psumbp.tile([P, S_max, H], f32, tag="drows")
                for c in range(S):
                    lhs = oht_t[:, c * P:(c + 1) * P]
                    nc.tensor.matmul(drows_ps[:, c, :], lhsT=lhs,
                                     rhs=ehi_t[:], start=True, stop=False)
                    nc.tensor.matmul(drows_ps[:, c, :], lhsT=lhs,
                                     rhs=elo_t[:], start=False, stop=True)

                # relrows chunks: accumulate one (hi+lo) mm pair per q
                # present in the chunk, via its parity one-hot tile
                for c in range(S):
                    rel_ps = psumrp.tile([P, H], f32, tag="relps")
                    qs = [q for (cc, q, _s0, _s1) in runs_all[b] if cc == c]
                    for i, q in enumerate(qs):
                        lhs = roht_t[q % n_par][:, c * P:(c + 1) * P]
                        nc.tensor.matmul(rel_ps[:], lhsT=lhs,
                                         rhs=rlh_t[:, q, :],
                                         start=(i == 0), stop=False)
                        nc.tensor.matmul(rel_ps[:], lhsT=lhs,
                                         rhs=rll_t[:, q, :],
                                         start=False, stop=(i == len(qs) - 1))
                    nc.scalar.copy(relrows[:, c, :], rel_ps[:])

                # comp (fp32, in-place over src_rows) + bf16 cast for accT
                nc.vector.tensor_tensor(
                    out=src_rows[:, 0:S, :], in0=src_rows[:, 0:S, :],
                    in1=relrows[:, 0:S, :], op=mybir.AluOpType.mult)
                nc.scalar.copy(comp_bf[:, 0:S, :], src_rows[:, 0:S, :])

                # score = sum_h comp*dstrows  (prod scratch into relrows)
                nc.vector.tensor_tensor(
                    out=relrows[:, 0:S, :], in0=src_rows[:, 0:S, :],
                    in1=drows_ps[:, 0:S, :], op=mybir.AluOpType.mult)
                score = smallp.tile([P, S_max], f32, tag="score")
                nc.vector.tensor_reduce(
                    out=score[:, 0:S], in_=relrows[:, 0:S, :],
                    axis=mybir.AxisListType.X, op=mybir.AluOpType.add)
                es = smallp.tile([P, S_max], f32, tag="es")
                nc.scalar.activation(
                    out=es[:, 0:S], in_=score[:, 0:S],
                    func=mybir.ActivationFunctionType.Exp)

                # W one-hot (fp32) * es -> bf16
                doh_ap = doh_t[:, coff:coff + S]
                doh_b = bc(doh_ap, [doh_ap.ap[0], doh_ap.ap[1], [0, H]])
                iota_ap = iota_t[:]
                iota_b = bc(iota_ap, [iota_ap.ap[0], [0, S], iota_ap.ap[1]])
                nc.vector.tensor_tensor(
                    out=w_oh[:, 0:S, :], in0=doh_b, in1=iota_b,
                    op=mybir.AluOpType.is_equal)
                es_ap = es[:, 0:S]
                es_b = bc(es_ap, [es_ap.ap[0], es_ap.ap[1], [0, H]])
                nc.vector.tensor_tensor(
                    out=w_bf[:, 0:S, :], in0=w_oh[:, 0:S, :], in1=es_b,
                    op=mybir.AluOpType.mult)

                # Wsum (bf16, DVE) -> den = Wsum.T @ ones (1 matmul)
                wsum = smallp.tile([P, P], bf16, tag="wsum")
                nc.scalar.copy(wsum[:], w_bf[:, 0, :])
                for c in range(1, S):
                    nc.vector.tensor_tensor(
                        out=wsum[:], in0=wsum[:], in1=w_bf[:, c, :],
                        op=mybir.AluOpType.add)
                ps_m = psump.tile([P, H], f32, tag="misc")
                nc.tensor.matmul(ps_m[:, 0:1], lhsT=wsum[:], rhs=ones_bf[:],
                                 start=True, stop=True)

                # accT[h,j] += comp_c.T @ W_c  (bf16)
                acct_ps = psump.tile([P, P], f32, tag="accT")
                for c in range(S):
                    nc.tensor.matmul(
                        acct_ps[:], lhsT=comp_bf[:, c, :], rhs=w_bf[:, c, :],
                        start=(c == 0), stop=(c == S - 1))

                acct_sb = smallp.tile([P, P], f32, tag="acct_sb")
                nc.scalar.copy(acct_sb[:], acct_ps[:])
                den_sb = smallp.tile([P, 1], f32, tag="den_sb")
                nc.vector.tensor_scalar_max(den_sb[:], ps_m[:, 0:1], 1e-30)
                rden = smallp.tile([P, 1], f32, tag="rden")
                nc.vector.reciprocal(rden[:], den_sb[:])

                nc.tensor.matmul(ps_m[:], lhsT=acct_sb[:], rhs=w_t[:],
                                 start=True, stop=True)
                out_sb = smallp.tile([P, H], f32, tag="out_sb")
                nc.scalar.activation(
                    out=out_sb[:], in_=ps_m[:],
                    func=mybir.ActivationFunctionType.Tanh, scale=rden[:])
                nc.sync.dma_start(out[base:base + nodes_b, :],
                                  out_sb[:nodes_b, :])
                coff += S

    nc.compile()
    return nc


def _idx_to_gather_layout(arr):
    a = arr.reshape(-1, 16).T.astype(np.int16)
    return np.tile(a, (8, 1))


def _prep_inputs(ent_emb, rel_emb, neigh_w, src, dst, rel_id):
    """Edges by dst block; rel-q-partitioned sections with core-uniform
    slot layout; build src gather idx + one-hot index maps."""
    import ml_dtypes
    src = np.asarray(src).astype(np.int64)
    dst = np.asarray(dst).astype(np.int64)
    rel_id = np.asarray(rel_id).astype(np.int64)
    n_blocks = (NPC + P - 1) // P

    order = np.argsort(dst, kind="stable")
    src_s, dst_s, rel_s = src[order], dst[order], rel_id[order]
    g_s = (dst_s // NPC) * n_blocks + (dst_s % NPC) // P
    n_gblocks = N_CORES * n_blocks
    bounds = np.searchsorted(g_s, np.arange(n_gblocks + 1))

    # per (core, block, section, q) edge lists
    # section 0 = src<LO_ROWS, 1 = src>=LO_ROWS
    per = {}
    for c in range(N_CORES):
        for b in range(n_blocks):
            e0, e1 = bounds[c * n_blocks + b], bounds[c * n_blocks + b + 1]
            s_g, d_g, r_g = src_s[e0:e1], dst_s[e0:e1], rel_s[e0:e1]
            sec = (s_g >= LO_ROWS).astype(np.int64)
            q_g = r_g // P
            for s in (0, 1):
                for q in range(NQ):
                    m = (sec == s) & (q_g == q)
                    per[(c, b, s, q)] = (s_g[m], d_g[m], r_g[m])

    # core-uniform slot counts per (block, section, q), 16-aligned for tidiness
    cnt = {}
    for b in range(n_blocks):
        for s in (0, 1):
            for q in range(NQ):
                m = max(len(per[(c, b, s, q)][0]) for c in range(N_CORES))
                if s == 0 and q == 0:
                    m = max(m, 1)
                cnt[(b, s, q)] = m

    # slot layout per block: lo qs, pad to 128; hi qs, pad to 128
    s_los, s_his, runs_all, layouts = [], [], [], []
    for b in range(n_blocks):
        lo_n = sum(cnt[(b, 0, q)] for q in range(NQ))
        hi_n = sum(cnt[(b, 1, q)] for q in range(NQ))
        s_lo = max((lo_n + P - 1) // P, 1)
        s_hi = (hi_n + P - 1) // P
        s_los.append(s_lo)
        s_his.append(s_hi)
        # slot ranges: [(q, s0, s1, sec)] in slot order; section pads get
        # q = last q of the section (repeat edges carry that q's relid)
        lay = []
        pos = 0
        for s, sbase, stot in ((0, 0, s_lo), (1, s_lo, s_hi)):
            pos = sbase * P
            for q in range(NQ):
                n = cnt[(b, s, q)]
                if n:
                    lay.append((q, pos, pos + n, s, False))
                    pos += n
            end = (sbase + stot) * P
            if pos < end and lay:
                lq, ls0, _ls1, lsec, lpad = lay[-1]
                if lsec == s and not lpad:
                    # extend the last real range; fill pads it with repeats
                    lay[-1] = (lq, ls0, end, s, False)
                else:
                    lay.append((lq, pos, end, s, True))
        layouts.append(lay)
        # runs: intersect layout ranges with 128-chunks
        runs = []
        for (q, s0, s1, _sec, _pad) in lay:
            c0, c1 = s0 // P, (s1 - 1) // P
            for c in range(c0, c1 + 1):
                a = max(s0, c * P)
                z = min(s1, (c + 1) * P)
                if a < z:
                    runs.append((c, q, a, z))
        runs_all.append(runs)
    s_tot = [a + b for a, b in zip(s_los, s_his)]
    W_chunks = sum(s_tot)

    iota = np.broadcast_to(np.arange(P, dtype=np.float32), (P, P)).copy()
    iota_col = np.arange(P, dtype=np.float32).reshape(P, 1).copy()
    iota_col16 = iota_col.astype(np.float16)

    # smallest parity count with no per-chunk q%k collision
    n_par = 3
    while True:
        ok = True
        for runs in runs_all:
            from collections import defaultdict
            byc = defaultdict(list)
            for (c, q, _s0, _s1) in runs:
                byc[c].append(q % n_par)
            if any(len(v) != len(set(v)) for v in byc.values()):
                ok = False
                break
        if ok:
            break
        n_par += 1
    rel_pad = np.zeros((NQ * P, H), np.float32)
    rel_pad[:N_REL] = np.asarray(rel_emb, np.float32)
    rhi, rlo = _bfsplit(rel_pad)
    rlh = np.ascontiguousarray(rhi.reshape(NQ, P, H).transpose(1, 0, 2))
    rll = np.ascontiguousarray(rlo.reshape(NQ, P, H).transpose(1, 0, 2))

    in_maps = []
    for cidx in range(N_CORES):
        sgi = np.zeros((W_chunks * P,), np.int16)
        doh = np.full((W_chunks * P,), float(P), np.float32)
        rid = np.zeros((W_chunks * P,), np.float16)
        qof = np.full((W_chunks * P,), -1, np.int64)
        coff = 0
        for b in range(n_blocks):
            o0 = coff * P
            for (q, s0, s1, sec, is_pad) in layouts[b]:
                ss, dd, rr = per[(cidx, b, sec, q)]
                if is_pad:
                    ss = ss[:0]
                    dd = dd[:0]
                    rr = rr[:0]
                n = len(ss)
                cap = s1 - s0
                assert n <= cap
                sub = LO_ROWS if sec == 1 else 0
                base = cidx * NPC + b * P
                qof[o0 + s0:o0 + s1] = q
                if n:
                    sgi[o0 + s0:o0 + s0 + n] = ss - sub
                    doh[o0 + s0:o0 + s0 + n] = (dd - base).astype(np.float32)
                    rid[o0 + s0:o0 + s0 + n] = (rr - q * P).astype(np.float16)
                if n < cap:
                    # pad: repeat a real edge of this q (doh stays 128),
                    # or r'=0 of this q if the core has none
                    sgi[o0 + s0 + n:o0 + s1] = (ss[0] - sub) if n else 0
                    rid[o0 + s0 + n:o0 + s1] = (rr[0] - q * P) if n else 0.0
            coff += s_los[b] + s_his[b]

        sgi_cols, doh_cols = [], []
        coff = 0
        for b in range(n_blocks):
            s_lo, s_hi, S = s_los[b], s_his[b], s_tot[b]
            o0 = coff * P
            lo_a = _idx_to_gather_layout(sgi[o0:o0 + s_lo * P])
            hi_a = (_idx_to_gather_layout(sgi[o0 + s_lo * P:o0 + S * P])
                    if s_hi > 0 else np.zeros((P, 0), np.int16))
            sgi_cols.append(np.concatenate([lo_a, hi_a], axis=1))
            doh_cols.append(doh[o0:o0 + S * P].reshape(S, P).T)
            coff += S
        sgi_l = np.concatenate(sgi_cols, axis=1)
        doh_l = np.concatenate(doh_cols, axis=1)
        dohT = np.broadcast_to(doh[None, :], (P, W_chunks * P))
        ridTs = []
        for p in range(n_par):
            rp = np.where(qof % n_par == p, rid.astype(np.float32),
                          -1.0).astype(np.float16)
            ridTs.append(np.ascontiguousarray(
                np.broadcast_to(rp[None, :], (P, W_chunks * P))))

        el = np.asarray(ent_emb, np.float32)[cidx * NPC:(cidx + 1) * NPC]
        ehi, elo = _bfsplit(el)

        in_maps.append({
            "ent": np.ascontiguousarray(ent_emb, np.float32),
            "ent_loc_hi": np.ascontiguousarray(ehi),
            "ent_loc_lo": np.ascontiguousarray(elo),
            "rel_hi": rlh,
            "rel_lo": rll,
            "w": np.ascontiguousarray(neigh_w, np.float32),
            "iota": iota,
            "iota_col": iota_col,
            "iota_col16": iota_col16,
            "src_gi": np.ascontiguousarray(sgi_l),
            "dst_oh": np.ascontiguousarray(doh_l),
            "dst_ohT": np.ascontiguousarray(dohT),
            **{f"relidT{p}": ridTs[p] for p in range(n_par)},
        })
    key = (NPC, N_ENT, LO_ROWS, tuple(s_los), tuple(s_his), n_par,
           tuple(tuple(r) for r in sum(runs_all, [])))
    return in_maps, key, s_los, s_his, runs_all, n_par


LAST_RESULT = None


def _install_ntff_hook():
    import sys
    import types
    if "antenv.axon_hooks" in sys.modules:
        return
    mod = types.ModuleType("antenv.axon_hooks")
    hook = [None]
    mod.set_axon_ntff_profile_hook = lambda h: hook.__setitem__(0, h)
    mod.get_axon_ntff_profile_hook = lambda: hook[0]
    sys.modules["antenv.axon_hooks"] = mod
    import antenv
    antenv.axon_hooks = mod
    try:
        from trn_agent_boot.trn_boot import _ntff_profile_via_ctypes
        h = _ntff_profile_via_ctypes("/opt/axon/libaxon_pjrt.so")
        if h is not None:
            mod.set_axon_ntff_profile_hook(lambda *a, **k: h(*a, **k))
    except Exception as e:
        print("ntff hook install failed:", e)


def kernel(ent_emb, rel_emb, neigh_w, src, dst, rel_id, _trace=False):
    global LAST_RESULT
    from concourse.bass_utils import run_bass_kernel_spmd
    if _trace:
        _install_ntff_hook()

    in_maps, key, s_los, s_his, runs_all, n_par = _prep_inputs(
        ent_emb, rel_emb, neigh_w, src, dst, rel_id)
    if key not in _cache:
        _cache[key] = _build_program(NPC, N_ENT, LO_ROWS,
                                     s_los, s_his, runs_all, n_par)
    nc = _cache[key]
    res = run_bass_kernel_spmd(nc, in_maps, list(range(N_CORES)),
                               trace=_trace)
    LAST_RESULT = res
    return np.concatenate([r["out"] for r in res.results], axis=0)
